# revision 4
# baseline (speedup 1.0000x reference)
"""GCN encoder (2-layer masked-attention message passing) on 8 TRN2 cores.

Data-parallel over batch B=8 -> 1 graph per NeuronCore; each core holds the
full (small) parameter set. Per-core design (N=2048, E=512, FF=256, L=2):

  - All five matmul groups (Q/K proj, scores, softmax-denominator, ctx,
    ctx2) run as fp8e4 DoubleRow matmuls: K=256 contraction per
    instruction at 0.5 cycles/row -- 4x f32r throughput. Weights are
    host-prescaled by 8 to keep fp8 out of subnormals; pT carries a x1
    natural scale, ctxT8 a x32 scale (via a 1/32 ones tile), and the
    residual fuses the combined 1/256 descale.
  - Masking is PE-side: a second DR matmul accumulates -240*cm (cm fp8,
    240 where no edge; stride-0 k-tile dim) into the scores PSUM, so
    exp(SCL*(s - 57600)) underflows to exactly 0 -- no DVE mask pass.
  - Softmax skips max-subtraction (scores ~N(0,0.2^2) after descale) and
    the denominator comes from a DR ones-matmul; ctxT normalization is
    fused into the PSUM->SBUF drain via the broadcast reciprocal.
  - Residual stream in bf16; LN via per-chunk bn_stats fused behind each
    residual add, DVE-Newton rsqrt, and a 4x-mode DVE apply; a Pool-side
    tensor_scalar produces the fp8 copy of x for the next layer's ctx.
  - x^T for layer 1 is rebuilt with bf16 PE transposes (fp8 transposes
    need stride-2 outputs) drained to fp8 on ACT.
  - Software-pipelined emission: phase A(ot) interleaves B(ot-1),
    C1(ot-1) (ctx2+residual+LN+applies) and the transposes of ot-2;
    layer-1 K-projections chase each node-slice's transposes so the
    layer boundary only exposes the last slice's chain. Engine split
    keeps ACT = exp metronome + ready PSUM copies, DVE = PSUM drains +
    LN, Pool = SBUF-only applies (GPSIMD cannot access PSUM).

TimelineSim 138.4us vs 275.4us baseline (1.99x); HW-verified median rel
err 3.2e-3 (fp8 path; gate 2e-2).
"""

import os
import sys

for _p in ("/root/.axon_site/_ro/trn_rl_repo", "/opt/trn_rl_repo"):
    if os.path.isdir(_p) and _p not in sys.path:
        sys.path.append(_p)

import numpy as np

B, N, E, FF, L = 8, 2048, 512, 256, 2
P = 128
NC = N // P      # 16 node chunks
EC = E // P      # 4 embed chunks
FC = FF // P     # 2 ff chunks
OW = 512         # o-tile width
OT = N // OW     # 4 o-tiles
OS = OW // P     # 4 o-subtiles per o-tile
WS = 8.0         # host prescale on Wq/Wk/Wc (keeps fp8 out of subnormals)
SCL = 1.0 / (np.sqrt(FF) * WS * WS)   # exp scale = 1/(16*64) = 2^-10
MK = 240.0       # seed magnitude (TRN fp8e4 max normal)
CTX_S = 32.0     # ctxT8 = 32 * ctxT  (ones tile = 1/32)
RES_S = 1.0 / (CTX_S * WS)            # ctx2 descale in residual = 1/256
LN_EPS = 1e-5

_CACHE = {}


def _build(apply_gb: bool, apply_bias: bool):
    import concourse.bass as bass
    import concourse.tile as tile
    from concourse import bacc, mybir

    f32 = mybir.dt.float32
    bf16 = mybir.dt.bfloat16
    f8 = mybir.dt.float8e4
    i32 = mybir.dt.int32
    AF = mybir.ActivationFunctionType
    ALU = mybir.AluOpType
    DR = mybir.MatmulPerfMode.DoubleRow

    nc = bacc.Bacc(
        "TRN2", target_bir_lowering=False, debug=False, num_devices=B
    )

    x0bf = nc.dram_tensor("x0bf", [N, E], bf16, kind="ExternalInput")
    x0t8 = nc.dram_tensor("x0t8", [E, N], f8, kind="ExternalInput")
    x08 = nc.dram_tensor("x08", [N, E], f8, kind="ExternalInput")
    wq8 = nc.dram_tensor("wq8", [L, E, FF], f8, kind="ExternalInput")
    wk8 = nc.dram_tensor("wk8", [L, E, FF], f8, kind="ExternalInput")
    wc8 = nc.dram_tensor("wc8", [L, E, E], f8, kind="ExternalInput")
    # cm8[p, ot, ic, j] = 240 where NO edge(o=ot*512+j <- i=ic*128+p)
    cm8 = nc.dram_tensor("cm8", [P, OT * NC * OW], f8, kind="ExternalInput")
    idn8 = nc.dram_tensor("idn8", [P, P], f8, kind="ExternalInput")
    idnb = nc.dram_tensor("idnb", [P, P], bf16, kind="ExternalInput")
    # seedw[:,0,:] = -240*I, [:,1,:] = 0
    seedwd = nc.dram_tensor("seedwd", [P, 2 * P], f8, kind="ExternalInput")
    onesd = nc.dram_tensor("onesd", [P, 2 * P], f8, kind="ExternalInput")
    if apply_bias:
        bqd = nc.dram_tensor("bqd", [L, FF], f32, kind="ExternalInput")
        bkd = nc.dram_tensor("bkd", [L, FF], f32, kind="ExternalInput")
    if apply_gb:
        ln_g = nc.dram_tensor("ln_g", [L, E], f32, kind="ExternalInput")
        ln_b = nc.dram_tensor("ln_b", [L, E], f32, kind="ExternalInput")
    outp = nc.dram_tensor("outp", [N, E], f32, kind="ExternalOutput")

    with tile.TileContext(nc) as tc:
        with (
            tc.tile_pool(name="persist", bufs=1) as persist,
            tc.tile_pool(name="pt", bufs=2) as ptpool,
            tc.tile_pool(name="ct", bufs=2) as ctpool,
            tc.tile_pool(name="rb", bufs=2) as rbpool,
            tc.tile_pool(name="ln", bufs=16) as lnpool,
            tc.tile_pool(name="of", bufs=4) as ofpool,
            tc.tile_pool(name="sc", bufs=2, space="PSUM") as scpool,
            tc.tile_pool(name="mid", bufs=2, space="PSUM") as midpool,
            tc.tile_pool(name="dn", bufs=2, space="PSUM") as dnpool,
        ):
            # ---------------- persistent SBUF ----------------
            XA = persist.tile([P, NC, E], bf16, tag="XA")   # layer-0 x (residual)
            XB = persist.tile([P, NC, E], bf16, tag="XB")   # layer-0 out / layer-1 x
            HB = persist.tile([P, NC, E], bf16, tag="HB")   # pre-LN h staging
            X8A = persist.tile([P, NC, E], f8, tag="X8A")   # fp8 x, layer 0
            X8B = persist.tile([P, NC, E], f8, tag="X8B")   # fp8 x, layer 1
            XT = persist.tile([P, EC, N], f8, tag="XT")     # x^T fp8
            # double-buffered per layer so layer-1 projections can run
            # while layer-0 attention still reads its Q/K
            QT = persist.tile([P, L, FC, N], f8, tag="QT")
            KT = persist.tile([P, L, FC, N], f8, tag="KT")
            CM = persist.tile([P, OT, NC, OW], f8, tag="CM")
            W8Q = persist.tile([P, L, EC, FF], f8, tag="W8Q")
            W8K = persist.tile([P, L, EC, FF], f8, tag="W8K")
            W8C = persist.tile([P, L, EC, E], f8, tag="W8C")
            ident = persist.tile([P, P], f8, tag="ident")
            identb = persist.tile([P, P], bf16, tag="identb")
            epsb = persist.tile([P, 1], f32, tag="epsb")
            seedw = persist.tile([P, 2, P], f8, tag="seedw")
            ones32 = persist.tile([P, 2, P], f8, tag="ones32")
            if apply_bias:
                bq_sb = persist.tile([P, L, FC], f32, tag="bq")
                bk_sb = persist.tile([P, L, FC], f32, tag="bk")
            g_sb = b_sb = None
            if apply_gb:
                g_sb = persist.tile([P, L, E], f32, tag="g")
                b_sb = persist.tile([P, L, E], f32, tag="b")

            def bcast_pair(src2d):
                # [P, X] AP -> [P, 2, X] with stride-0 middle (k-tile) dim
                return bass.AP(
                    tensor=src2d.tensor,
                    offset=src2d.offset,
                    ap=[src2d.ap[0], [0, 2], src2d.ap[-1]],
                )

            def bcast_part(src):  # broadcast a DRAM AP across partitions
                return bass.AP(
                    tensor=src.tensor, offset=src.offset, ap=[[0, P], *src.ap]
                )

            # ---------------- DMA loads (issue order matters) ----------------
            nc.vector.memset(epsb[:, :], LN_EPS)
            nc.gpsimd.dma_start(out=identb, in_=idnb[:, :])
            nc.gpsimd.dma_start(out=ident, in_=idn8[:, :])
            nc.gpsimd.dma_start(
                out=seedw, in_=seedwd.rearrange("p (t q) -> p t q", t=2)
            )
            nc.gpsimd.dma_start(
                out=ones32, in_=onesd.rearrange("p (t q) -> p t q", t=2)
            )
            if apply_bias:
                nc.gpsimd.dma_start(
                    out=bq_sb, in_=bqd.rearrange("l (c p) -> p l c", p=P)
                )
                nc.gpsimd.dma_start(
                    out=bk_sb, in_=bkd.rearrange("l (c p) -> p l c", p=P)
                )
            if apply_gb:
                nc.gpsimd.dma_start(out=g_sb, in_=bcast_part(ln_g[:, :]))
                nc.gpsimd.dma_start(out=b_sb, in_=bcast_part(ln_b[:, :]))

            nc.sync.dma_start(
                out=W8K, in_=wk8.rearrange("l (c p) f -> p l c f", p=P)
            )
            nc.sync.dma_start(
                out=W8Q, in_=wq8.rearrange("l (c p) f -> p l c f", p=P)
            )
            cmr = cm8.rearrange("p (o c w) -> p o c w", o=OT, c=NC)
            # XT node-slices interleaved with CM(ot=0) quarters: scores(0,g)
            # needs only XT/proj up to slice g//2 and CM quarter g//2
            x0tr = x0t8.rearrange("(c p) n -> p c n", p=P)
            x08r = x08.rearrange("(c p) e -> p c e", p=P)
            nc.sync.dma_start(out=XT[:, :, 0:OW], in_=x0tr[:, :, 0:OW])
            for q in range(4):
                nc.sync.dma_start(
                    out=CM[:, 0, 4 * q : 4 * q + 4, :],
                    in_=cmr[:, 0, 4 * q : 4 * q + 4, :],
                )
                if q < 3:
                    nt = q + 1
                    nc.sync.dma_start(
                        out=XT[:, :, nt * OW : (nt + 1) * OW],
                        in_=x0tr[:, :, nt * OW : (nt + 1) * OW],
                    )
            nc.sync.dma_start(out=X8A, in_=x08r)
            nc.sync.dma_start(out=CM[:, 1], in_=cmr[:, 1])
            nc.sync.dma_start(
                out=XA, in_=x0bf.rearrange("(c p) e -> p c e", p=P)
            )
            nc.sync.dma_start(
                out=W8C, in_=wc8.rearrange("l (c p) e -> p l c e", p=P)
            )
            for ot in range(2, OT):
                nc.sync.dma_start(out=CM[:, ot], in_=cmr[:, ot])
            outr = outp.rearrange("(c p) e -> p c e", p=P)

            # PE warm-up: burn the 1.2GHz clock-gate window on dummy
            # transposes of the identity (arrives first via SWDGE).
            warm0 = midpool.tile([P, OW], f32, tag="mid")
            warmps = warm0.bitcast(bf16)
            for _ in range(12):
                nc.tensor.matmul(
                    warmps[:, 0:P],
                    lhsT=identb,
                    rhs=identb,
                    is_transpose=True,
                    start=True,
                    stop=True,
                    skip_group_check=True,
                )

            # ---------------- per-layer state ----------------
            def mk_tp(Xsrc, ot, ec):
                # transpose 4 bf16 node chunks of Xsrc; the drain converts
                # to fp8 XT (fp8 PE transposes need stride-2 outputs, so
                # transpose in bf16 instead)
                def op():
                    oc0 = ot * OS
                    tp0 = midpool.tile([P, OW], f32, tag="mid")
                    tps = tp0.bitcast(bf16)[:, 0:OW]
                    for j in range(OS):
                        nc.tensor.matmul(
                            tps[:, j * P : (j + 1) * P],
                            lhsT=Xsrc[:, oc0 + j, ec * P : (ec + 1) * P],
                            rhs=identb,
                            is_transpose=True,
                            start=True,
                            stop=True,
                            skip_group_check=True,
                        )
                    nc.scalar.copy(XT[:, ec, ot * OW : (ot + 1) * OW], tps)
                return op

            def mk_proj(k, di, nt):
                # one (dst, node-slice) projection: 4 DR matmuls + drain
                dst, w8, b_sb2 = (
                    (KT, W8K, bk_sb if apply_bias else None),
                    (QT, W8Q, bq_sb if apply_bias else None),
                )[di]

                def op():
                    ps = scpool.tile([P, 2, OW], f32, tag="sc")
                    for fc in range(FC):
                        for e2 in range(2):
                            nc.tensor.matmul(
                                ps[:, fc, :],
                                lhsT=w8[
                                    :, k, 2 * e2 : 2 * e2 + 2,
                                    fc * P : (fc + 1) * P,
                                ],
                                rhs=XT[
                                    :, 2 * e2 : 2 * e2 + 2,
                                    nt * OW : (nt + 1) * OW,
                                ],
                                start=(e2 == 0),
                                stop=(e2 == 1),
                                perf_mode=DR,
                            )
                    dstap = dst[:, k, :, nt * OW : (nt + 1) * OW]
                    if apply_bias:
                        for fc in range(FC):
                            if (di + nt) % 2 == 0:
                                nc.scalar.add(
                                    dstap[:, fc, :],
                                    ps[:, fc, :],
                                    b_sb2[:, k, fc : fc + 1],
                                )
                            else:
                                nc.vector.tensor_scalar(
                                    out=dstap[:, fc, :],
                                    in0=ps[:, fc, :],
                                    scalar1=b_sb2[:, k, fc : fc + 1],
                                    scalar2=None,
                                    op0=ALU.add,
                                )
                    elif (di + nt) % 2 == 0:
                        nc.scalar.copy(dstap, ps)
                    else:
                        nc.vector.tensor_copy(dstap, ps)
                return op

            def projections(k):
                # slice 0 inline; the rest feed phase_a(k, 0)'s interleave
                # so scores can start after just K0/Q0
                mk_proj(k, 0, 0)()
                mk_proj(k, 1, 0)()
                return [
                    mk_proj(k, di, nt)
                    for nt in range(1, OT)
                    for di in range(2)
                ]

            # phase A for one ot: scores+seed -> exp -> dn (dn lags one
            # group so PE never waits on the exp that feeds it)
            def phase_a(k, ot, pT, dn_tile, interleave, drain_by=None,
                        lazy=()):
                osl = slice(ot * OW, (ot + 1) * OW)
                NG = NC // 2
                DB = drain_by or NG
                ii = 0
                jj = 0

                def dn_mm(g):
                    nc.tensor.matmul(
                        dn_tile,
                        lhsT=ones32,
                        rhs=pT[:, 2 * g : 2 * g + 2, :],
                        start=(g == 0),
                        stop=(g == NG - 1),
                        perf_mode=DR,
                    )

                for g in range(NG):
                    sc = scpool.tile([P, 2, OW], f32, tag="sc")
                    for h in range(2):
                        ic = 2 * g + h
                        nc.tensor.matmul(
                            sc[:, h, :],
                            lhsT=KT[:, k, 0:2, ic * P : (ic + 1) * P],
                            rhs=QT[:, k, 0:2, osl],
                            start=True,
                            stop=False,
                            perf_mode=DR,
                        )
                        nc.tensor.matmul(
                            sc[:, h, :],
                            lhsT=seedw,
                            rhs=bcast_pair(CM[:, ot, ic, :]),
                            start=False,
                            stop=True,
                            perf_mode=DR,
                        )
                    nc.scalar.activation(
                        pT[:, 2 * g : 2 * g + 2, :], sc, AF.Exp, scale=SCL
                    )
                    if g > 0:
                        dn_mm(g - 1)
                    # interleave pending ops from prev-ot phases B/C, paced
                    # so the list drains evenly across the first DB groups
                    quota = ((g + 1) * len(interleave) + DB - 1) // DB
                    while ii < min(quota, len(interleave)):
                        interleave[ii]()
                        ii += 1
                    if g >= NG - 3 and lazy:
                        ql = ((g - NG + 4) * len(lazy) + 2) // 3
                        while jj < min(ql, len(lazy)):
                            lazy[jj]()
                            jj += 1
                dn_mm(NG - 1)
                while ii < len(interleave):
                    interleave[ii]()
                    ii += 1
                while jj < len(lazy):
                    lazy[jj]()
                    jj += 1

            # phase B for one ot -> returns list of emission thunks
            def phase_b_ops(k, ot, pT, ctxT8, rb):
                X8 = X8A if k == 0 else X8B

                def mk_ctx(ec):
                    def op():
                        cp = midpool.tile([P, OW], f32, tag="mid")
                        for j in range(NC // 2):
                            nc.tensor.matmul(
                                cp,
                                lhsT=X8[
                                    :, 2 * j : 2 * j + 2,
                                    ec * P : (ec + 1) * P,
                                ],
                                rhs=pT[:, 2 * j : 2 * j + 2, :],
                                start=(j == 0),
                                stop=(j == NC // 2 - 1),
                                perf_mode=DR,
                            )
                        nc.vector.tensor_tensor(
                            ctxT8[:, ec, :], cp, rb[:, :], ALU.mult
                        )
                    return op

                if os.environ.get("ABL_B"):
                    return []
                return [mk_ctx(ec) for ec in range(EC)]

            # phase C for one ot -> list of emission thunks
            def phase_c_ops(k, ot, ctxT8):
                X_in = XA if k == 0 else XB
                X_out = XB if k == 0 else None
                mv = lnpool.tile([P, OS, 2], f32, tag="mv")
                stats = lnpool.tile([P, OS, 6], f32, tag="st")
                y4 = lnpool.tile([P, OS], f32, tag="y4")

                def mk_ctx2(osub):
                    def op():
                        cp = midpool.tile([P, OW], f32, tag="mid")
                        for e2 in range(2):
                            nc.tensor.matmul(
                                cp,
                                lhsT=ctxT8[
                                    :, 2 * e2 : 2 * e2 + 2,
                                    osub * P : (osub + 1) * P,
                                ],
                                rhs=W8C[:, k, 2 * e2 : 2 * e2 + 2, :],
                                start=(e2 == 0),
                                stop=(e2 == 1),
                                perf_mode=DR,
                            )
                        oc = ot * OS + osub
                        # h = ctx2 * RES_S + x  (bf16 staging; DVE --
                        # GPSIMD cannot read the ctx2 PSUM)
                        nc.vector.scalar_tensor_tensor(
                            HB[:, oc, :],
                            cp,
                            RES_S,
                            X_in[:, oc, :],
                            ALU.mult,
                            ALU.add,
                        )
                        # per-chunk LN stats immediately (shortens the tail)
                        nc.vector.bn_stats(stats[:, osub, :], HB[:, oc, :])
                        nc.vector.bn_aggr(mv[:, osub, :], stats[:, osub, :])
                    return op

                def ln_stats():
                    # rstd via Newton on [P, OS] (keeps the dependent
                    # chain off the in-order ACT exp stream)
                    x4 = lnpool.tile([P, OS], f32, tag="x4")
                    t4 = lnpool.tile([P, OS], f32, tag="t4")
                    nc.vector.tensor_scalar_add(x4, mv[:, :, 1], LN_EPS)
                    nc.vector.tensor_scalar(
                        out=y4.bitcast(i32),
                        in0=x4.bitcast(i32),
                        scalar1=1,
                        scalar2=None,
                        op0=ALU.logical_shift_right,
                    )
                    nc.vector.tensor_scalar(
                        out=y4.bitcast(i32),
                        in0=y4.bitcast(i32),
                        scalar1=-1,
                        scalar2=0x5F3759DF,
                        op0=ALU.mult,
                        op1=ALU.add,
                    )
                    for _ in range(2):
                        nc.vector.tensor_mul(t4, y4, y4)
                        nc.vector.tensor_mul(t4, t4, x4)
                        nc.vector.tensor_scalar(
                            out=t4,
                            in0=t4,
                            scalar1=-0.5,
                            scalar2=1.5,
                            op0=ALU.mult,
                            op1=ALU.add,
                        )
                        nc.vector.tensor_mul(y4, y4, t4)

                def mk_apply(osub):
                    def op():
                        oc = ot * OS + osub
                        # x_out (bf16, DVE 4x)
                        nc.vector.tensor_scalar(
                            out=X_out[:, oc, :],
                            in0=HB[:, oc, :],
                            scalar1=mv[:, osub, 0:1],
                            scalar2=y4[:, osub : osub + 1],
                            op0=ALU.subtract,
                            op1=ALU.mult,
                        )
                        if apply_gb:
                            nc.gpsimd.tensor_mul(
                                X_out[:, oc, :], X_out[:, oc, :],
                                g_sb[:, k, :],
                            )
                            nc.gpsimd.tensor_add(
                                X_out[:, oc, :], X_out[:, oc, :],
                                b_sb[:, k, :],
                            )
                            nc.gpsimd.tensor_copy(
                                X8B[:, oc, :], X_out[:, oc, :]
                            )
                        else:
                            nc.gpsimd.tensor_scalar(
                                out=X8B[:, oc, :],
                                in0=HB[:, oc, :],
                                scalar1=mv[:, osub, 0:1],
                                scalar2=y4[:, osub : osub + 1],
                                op0=ALU.subtract,
                                op1=ALU.mult,
                            )
                    return op

                def mk_final(osub):
                    def op():
                        ops_ps = ofpool.tile([P, E], f32, tag="of")
                        oc = ot * OS + osub
                        eng = nc.gpsimd if osub % 2 else nc.vector
                        eng.tensor_scalar(
                            out=ops_ps,
                            in0=HB[:, oc, :],
                            scalar1=mv[:, osub, 0:1],
                            scalar2=y4[:, osub : osub + 1],
                            op0=ALU.subtract,
                            op1=ALU.mult,
                        )
                        if apply_gb:
                            nc.gpsimd.tensor_mul(
                                ops_ps, ops_ps, g_sb[:, k, :]
                            )
                            nc.gpsimd.tensor_add(
                                ops_ps, ops_ps, b_sb[:, k, :]
                            )
                        nc.sync.dma_start(out=outr[:, oc, :], in_=ops_ps)
                    return op

                # c1: consumed during phase_a(ot+1) -- full LN chain so the
                # transposes (c2, phase_a(ot+2)) never stall on applies
                if os.environ.get("ABL_C"):
                    return [], []
                c1 = [mk_ctx2(i) for i in range(OS)] + [ln_stats]
                if k < L - 1:
                    c1 += [mk_apply(i) for i in range(OS)]
                    c2 = ([mk_tp(X_out, ot, ec) for ec in range(EC)]
                          if not os.environ.get("ABL_TP") else [])
                else:
                    c1 += [mk_final(i) for i in range(OS)]
                    c2 = []
                return c1, c2

            # ---------------- main schedule ----------------
            # pipeline: A(k,ot) interleaves B(ot-1) + C1(ot-1) + tp(ot-2);
            # projK(k+1, nt) right after A(k, nt+2); projQ(k+1, nt) lazily
            # just before A(k+1, nt). Pendings carry across the layer
            # boundary (chain-first, forced-early pacing).
            pend = projections(0)
            b_prev = []     # B(ot-1)
            c1_prev = []    # C1(ot-1): ctx2+residual+LN+applies
            c2_p = []       # tp(ot-1)
            c2_pp = []      # tp(ot-2)
            projk_p = []    # projK(k+1, ot-1)
            projk_pp = []   # projK(k+1, ot-2)
            projq = {}      # nt -> projQ(k+1, nt) thunk
            for k in range(L):
                for ot in range(OT):
                    boundary = ot == 0 and k > 0
                    if boundary:
                        # layer tail: ready transposes/projK first, then
                        # the dependent chain; drain early so projK(3)
                        # precedes scores g6 (which reads KT's last slice)
                        inter = (c2_pp + projk_pp + b_prev + c1_prev
                                 + c2_p + projk_p)
                        lz = []
                        c2_pp = c2_p = projk_pp = projk_p = []
                        db = 5
                    else:
                        inter = pend + b_prev + c1_prev + c2_pp
                        lz = []
                        c2_pp = []
                        db = 6 if (k == 0 and ot == 0) else None
                    pend = []
                    if k > 0 and ot in projq:
                        projq.pop(ot)()
                    pT = ptpool.tile([P, NC, OW], f8, tag="pT")
                    dn = dnpool.tile([P, OW], f32, tag="dn")
                    phase_a(k, ot, pT, dn[:, :], inter, drain_by=db,
                            lazy=lz)
                    # projK(k+1, nt=ot-2): its transposes (tp(ot-2)) were
                    # consumed inside phase_a above
                    for op in projk_pp:
                        op()
                    projk_pp = []
                    rb = rbpool.tile([P, OW], f32, tag="rb")
                    nc.vector.reciprocal(rb, dn[:, :])
                    ctxT8 = ctpool.tile([P, EC, OW], f8, tag="ctxT8")
                    b_prev = phase_b_ops(k, ot, pT, ctxT8, rb)
                    c1_prev, c2_new = phase_c_ops(k, ot, ctxT8)
                    c2_pp, c2_p = c2_p, c2_new
                    projk_pp = projk_p
                    if k < L - 1:
                        projk_p = [mk_proj(k + 1, 0, ot)]
                        projq[ot] = mk_proj(k + 1, 1, ot)
                    else:
                        projk_p = []
            # final flush after the last ot of the last layer
            for op in c2_pp + projk_pp + b_prev + c1_prev + c2_p + projk_p:
                op()
    nc.compile()
    return nc


def _get_nc(apply_gb, apply_bias):
    key = ("nc", apply_gb, apply_bias)
    if key not in _CACHE:
        _CACHE[key] = _build(apply_gb, apply_bias)
    return _CACHE[key]


def make_in_maps(inputs, apply_gb=None, apply_bias=None):
    import ml_dtypes

    f8 = ml_dtypes.float8_e4m3
    bf = ml_dtypes.bfloat16

    node_fts = np.asarray(inputs["node_fts"], np.float32)
    rel_edges = np.asarray(inputs["rel_edges"])
    Wq = np.asarray(inputs["Wq"], np.float32)
    bq = np.asarray(inputs["bq"], np.float32)
    Wk = np.asarray(inputs["Wk"], np.float32)
    bk = np.asarray(inputs["bk"], np.float32)
    Wc = np.asarray(inputs["Wc"], np.float32)
    ln_g = np.asarray(inputs["ln_g"], np.float32)
    ln_b = np.asarray(inputs["ln_b"], np.float32)
    if apply_gb is None:
        apply_gb = _needs_gb(inputs)
    if apply_bias is None:
        apply_bias = _needs_bias(inputs)

    wq8 = np.ascontiguousarray(
        (WS * Wq).transpose(0, 2, 1)).astype(f8)          # [L, E, FF]
    wk8 = np.ascontiguousarray((WS * Wk).transpose(0, 2, 1)).astype(f8)
    wc8 = np.ascontiguousarray((WS * Wc).transpose(0, 2, 1)).astype(f8)
    idn8 = np.eye(P, dtype=np.float32).astype(f8)
    seedw = np.zeros((P, 2 * P), np.float32)
    seedw[:, :P] = -MK * np.eye(P)
    seedw = seedw.astype(f8)
    ones = np.full((P, 2 * P), 1.0 / CTX_S, np.float32).astype(f8)

    in_maps = []
    for c in range(B):
        x = node_fts[c]
        # cm8[p, ot, ic, j] = MK where no edge(o <- i), i=ic*128+p, o=ot*512+j
        noedge = (rel_edges[c] == 0)                       # [o, i]
        cm = noedge.T.reshape(NC, P, OT, OW).transpose(1, 2, 0, 3)
        cm8 = np.ascontiguousarray(
            cm.astype(np.float32) * MK).astype(f8).reshape(P, -1)
        m = {
            "x0bf": np.ascontiguousarray(x).astype(bf),
            "x08": np.ascontiguousarray(x).astype(f8),
            "x0t8": np.ascontiguousarray(x.T).astype(f8),
            "wq8": wq8,
            "wk8": wk8,
            "wc8": wc8,
            "cm8": cm8,
            "idn8": idn8,
            "idnb": np.eye(P, dtype=np.float32).astype(bf),
            "seedwd": seedw,
            "onesd": ones,
        }
        if apply_bias:
            m["bqd"] = WS * bq
            m["bkd"] = WS * bk
        if apply_gb:
            m["ln_g"] = ln_g
            m["ln_b"] = ln_b
        in_maps.append(m)
    return in_maps


def _needs_gb(inputs):
    g = np.asarray(inputs["ln_g"], np.float32)
    b = np.asarray(inputs["ln_b"], np.float32)
    return not (np.all(g == 1.0) and np.all(b == 0.0))


def _needs_bias(inputs):
    bq = np.asarray(inputs["bq"], np.float32)
    bk = np.asarray(inputs["bk"], np.float32)
    return not (np.all(bq == 0.0) and np.all(bk == 0.0))


def kernel(**inputs) -> np.ndarray:
    from concourse.bass_utils import run_bass_kernel_spmd

    apply_gb = _needs_gb(inputs)
    apply_bias = _needs_bias(inputs)
    nc = _get_nc(apply_gb, apply_bias)
    in_maps = make_in_maps(inputs, apply_gb, apply_bias)
    res = run_bass_kernel_spmd(nc, in_maps, core_ids=list(range(B)))
    return np.stack([r["outp"] for r in res.results], axis=0)


# revision 5
# speedup vs baseline: 1.0306x; 1.0306x over previous
"""GCN encoder (2-layer masked-attention message passing) on 8 TRN2 cores.

Data-parallel over batch B=8 -> 1 graph per NeuronCore; each core holds the
full (small) parameter set. Per-core design (N=2048, E=512, FF=256, L=2):

  - All five matmul groups (Q/K proj, scores, softmax-denominator, ctx,
    ctx2) run as fp8e4 DoubleRow matmuls: K=256 contraction per
    instruction at 0.5 cycles/row -- 4x f32r throughput. Weights are
    host-prescaled by 8 to keep fp8 out of subnormals; pT carries a x1
    natural scale, ctxT8 a x32 scale (via a 1/32 ones tile), and the
    residual fuses the combined 1/256 descale.
  - Masking is PE-side: a second DR matmul accumulates -240*cm (cm fp8,
    240 where no edge; stride-0 k-tile dim) into the scores PSUM, so
    exp(SCL*(s - 57600)) underflows to exactly 0 -- no DVE mask pass.
  - Softmax skips max-subtraction (scores ~N(0,0.2^2) after descale) and
    the denominator comes from a DR ones-matmul; ctxT normalization is
    fused into the PSUM->SBUF drain via the broadcast reciprocal.
  - Residual stream in bf16; LN via per-chunk bn_stats fused behind each
    residual add, DVE-Newton rsqrt, and a 4x-mode DVE apply; a Pool-side
    tensor_scalar produces the fp8 copy of x for the next layer's ctx.
  - x^T for layer 1 is rebuilt with bf16 PE transposes (fp8 transposes
    need stride-2 outputs) drained to fp8 on ACT.
  - Software-pipelined emission: phase A(ot) interleaves B(ot-1),
    C1(ot-1) (ctx2+residual+LN+applies) and the transposes of ot-2;
    layer-1 K-projections chase each node-slice's transposes so the
    layer boundary only exposes the last slice's chain. Engine split
    keeps ACT = exp metronome + ready PSUM copies, DVE = PSUM drains +
    LN, Pool = SBUF-only applies (GPSIMD cannot access PSUM).

TimelineSim 134.3us vs 275.4us baseline (2.05x); HW-verified median rel
err 3.2e-3 (fp8 path; gate 2e-2).
"""

import os
import sys

for _p in ("/root/.axon_site/_ro/trn_rl_repo", "/opt/trn_rl_repo"):
    if os.path.isdir(_p) and _p not in sys.path:
        sys.path.append(_p)

import numpy as np

B, N, E, FF, L = 8, 2048, 512, 256, 2
P = 128
NC = N // P      # 16 node chunks
EC = E // P      # 4 embed chunks
FC = FF // P     # 2 ff chunks
OW = 512         # o-tile width
OT = N // OW     # 4 o-tiles
OS = OW // P     # 4 o-subtiles per o-tile
WS = 8.0         # host prescale on Wq/Wk/Wc (keeps fp8 out of subnormals)
SCL = 1.0 / (np.sqrt(FF) * WS * WS)   # exp scale = 1/(16*64) = 2^-10
MK = 240.0       # seed magnitude (TRN fp8e4 max normal)
CTX_S = 32.0     # ctxT8 = 32 * ctxT  (ones tile = 1/32)
RES_S = 1.0 / (CTX_S * WS)            # ctx2 descale in residual = 1/256
LN_EPS = 1e-5

_CACHE = {}


def _build(apply_gb: bool, apply_bias: bool):
    import concourse.bass as bass
    import concourse.tile as tile
    from concourse import bacc, mybir

    f32 = mybir.dt.float32
    bf16 = mybir.dt.bfloat16
    f8 = mybir.dt.float8e4
    i32 = mybir.dt.int32
    AF = mybir.ActivationFunctionType
    ALU = mybir.AluOpType
    DR = mybir.MatmulPerfMode.DoubleRow

    nc = bacc.Bacc(
        "TRN2", target_bir_lowering=False, debug=False, num_devices=B
    )

    x0bf = nc.dram_tensor("x0bf", [N, E], bf16, kind="ExternalInput")
    x0t8 = nc.dram_tensor("x0t8", [E, N], f8, kind="ExternalInput")
    x08 = nc.dram_tensor("x08", [N, E], f8, kind="ExternalInput")
    wq8 = nc.dram_tensor("wq8", [L, E, FF], f8, kind="ExternalInput")
    wk8 = nc.dram_tensor("wk8", [L, E, FF], f8, kind="ExternalInput")
    wc8 = nc.dram_tensor("wc8", [L, E, E], f8, kind="ExternalInput")
    # cm8[p, ot, ic, j] = 240 where NO edge(o=ot*512+j <- i=ic*128+p)
    cm8 = nc.dram_tensor("cm8", [P, OT * NC * OW], f8, kind="ExternalInput")
    idn8 = nc.dram_tensor("idn8", [P, P], f8, kind="ExternalInput")
    idnb = nc.dram_tensor("idnb", [P, P], bf16, kind="ExternalInput")
    # seedw[:,0,:] = -240*I, [:,1,:] = 0
    seedwd = nc.dram_tensor("seedwd", [P, 2 * P], f8, kind="ExternalInput")
    onesd = nc.dram_tensor("onesd", [P, 2 * P], f8, kind="ExternalInput")
    if apply_bias:
        bqd = nc.dram_tensor("bqd", [L, FF], f32, kind="ExternalInput")
        bkd = nc.dram_tensor("bkd", [L, FF], f32, kind="ExternalInput")
    if apply_gb:
        ln_g = nc.dram_tensor("ln_g", [L, E], f32, kind="ExternalInput")
        ln_b = nc.dram_tensor("ln_b", [L, E], f32, kind="ExternalInput")
    outp = nc.dram_tensor("outp", [N, E], f32, kind="ExternalOutput")

    with tile.TileContext(nc) as tc:
        with (
            tc.tile_pool(name="persist", bufs=1) as persist,
            tc.tile_pool(name="pt", bufs=2) as ptpool,
            tc.tile_pool(name="ct", bufs=2) as ctpool,
            tc.tile_pool(name="rb", bufs=2) as rbpool,
            tc.tile_pool(name="ln", bufs=16) as lnpool,
            tc.tile_pool(name="of", bufs=4) as ofpool,
            tc.tile_pool(name="sc", bufs=2, space="PSUM") as scpool,
            tc.tile_pool(name="mid", bufs=3, space="PSUM") as midpool,
            tc.tile_pool(name="dn", bufs=1, space="PSUM") as dnpool,
        ):
            # ---------------- persistent SBUF ----------------
            XA = persist.tile([P, NC, E], bf16, tag="XA")   # layer-0 x (residual)
            XB = persist.tile([P, NC, E], bf16, tag="XB")   # layer-0 out / layer-1 x
            HB = persist.tile([P, NC, E], bf16, tag="HB")   # pre-LN h staging
            X8A = persist.tile([P, NC, E], f8, tag="X8A")   # fp8 x, layer 0
            X8B = persist.tile([P, NC, E], f8, tag="X8B")   # fp8 x, layer 1
            XT = persist.tile([P, EC, N], f8, tag="XT")     # x^T fp8
            # double-buffered per layer so layer-1 projections can run
            # while layer-0 attention still reads its Q/K
            QT = persist.tile([P, L, FC, N], f8, tag="QT")
            KT = persist.tile([P, L, FC, N], f8, tag="KT")
            CM = persist.tile([P, OT, NC, OW], f8, tag="CM")
            W8Q = persist.tile([P, L, EC, FF], f8, tag="W8Q")
            W8K = persist.tile([P, L, EC, FF], f8, tag="W8K")
            W8C = persist.tile([P, L, EC, E], f8, tag="W8C")
            ident = persist.tile([P, P], f8, tag="ident")
            identb = persist.tile([P, P], bf16, tag="identb")
            epsb = persist.tile([P, 1], f32, tag="epsb")
            seedw = persist.tile([P, 2, P], f8, tag="seedw")
            ones32 = persist.tile([P, 2, P], f8, tag="ones32")
            if apply_bias:
                bq_sb = persist.tile([P, L, FC], f32, tag="bq")
                bk_sb = persist.tile([P, L, FC], f32, tag="bk")
            g_sb = b_sb = None
            if apply_gb:
                g_sb = persist.tile([P, L, E], f32, tag="g")
                b_sb = persist.tile([P, L, E], f32, tag="b")

            def bcast_pair(src2d):
                # [P, X] AP -> [P, 2, X] with stride-0 middle (k-tile) dim
                return bass.AP(
                    tensor=src2d.tensor,
                    offset=src2d.offset,
                    ap=[src2d.ap[0], [0, 2], src2d.ap[-1]],
                )

            def bcast_part(src):  # broadcast a DRAM AP across partitions
                return bass.AP(
                    tensor=src.tensor, offset=src.offset, ap=[[0, P], *src.ap]
                )

            # ---------------- DMA loads (issue order matters) ----------------
            nc.vector.memset(epsb[:, :], LN_EPS)
            nc.gpsimd.dma_start(out=identb, in_=idnb[:, :])
            nc.gpsimd.dma_start(out=ident, in_=idn8[:, :])
            nc.gpsimd.dma_start(
                out=seedw, in_=seedwd.rearrange("p (t q) -> p t q", t=2)
            )
            nc.gpsimd.dma_start(
                out=ones32, in_=onesd.rearrange("p (t q) -> p t q", t=2)
            )
            if apply_bias:
                nc.gpsimd.dma_start(
                    out=bq_sb, in_=bqd.rearrange("l (c p) -> p l c", p=P)
                )
                nc.gpsimd.dma_start(
                    out=bk_sb, in_=bkd.rearrange("l (c p) -> p l c", p=P)
                )
            if apply_gb:
                nc.gpsimd.dma_start(out=g_sb, in_=bcast_part(ln_g[:, :]))
                nc.gpsimd.dma_start(out=b_sb, in_=bcast_part(ln_b[:, :]))

            nc.sync.dma_start(
                out=W8K, in_=wk8.rearrange("l (c p) f -> p l c f", p=P)
            )
            nc.sync.dma_start(
                out=W8Q, in_=wq8.rearrange("l (c p) f -> p l c f", p=P)
            )
            cmr = cm8.rearrange("p (o c w) -> p o c w", o=OT, c=NC)
            # XT node-slices interleaved with CM(ot=0) quarters: scores(0,g)
            # needs only XT/proj up to slice g//2 and CM quarter g//2
            x0tr = x0t8.rearrange("(c p) n -> p c n", p=P)
            x08r = x08.rearrange("(c p) e -> p c e", p=P)
            nc.sync.dma_start(out=XT[:, :, 0:OW], in_=x0tr[:, :, 0:OW])
            for q in range(4):
                nc.sync.dma_start(
                    out=CM[:, 0, 4 * q : 4 * q + 4, :],
                    in_=cmr[:, 0, 4 * q : 4 * q + 4, :],
                )
                if q < 3:
                    nt = q + 1
                    nc.sync.dma_start(
                        out=XT[:, :, nt * OW : (nt + 1) * OW],
                        in_=x0tr[:, :, nt * OW : (nt + 1) * OW],
                    )
            nc.sync.dma_start(out=X8A, in_=x08r)
            nc.sync.dma_start(out=CM[:, 1], in_=cmr[:, 1])
            nc.sync.dma_start(
                out=XA, in_=x0bf.rearrange("(c p) e -> p c e", p=P)
            )
            nc.sync.dma_start(
                out=W8C, in_=wc8.rearrange("l (c p) e -> p l c e", p=P)
            )
            for ot in range(2, OT):
                nc.sync.dma_start(out=CM[:, ot], in_=cmr[:, ot])
            outr = outp.rearrange("(c p) e -> p c e", p=P)

            # PE warm-up: burn the 1.2GHz clock-gate window on dummy
            # transposes of the identity (arrives first via SWDGE).
            warm0 = midpool.tile([P, OW], f32, tag="mid")
            warmps = warm0.bitcast(bf16)
            for _ in range(12):
                nc.tensor.matmul(
                    warmps[:, 0:P],
                    lhsT=identb,
                    rhs=identb,
                    is_transpose=True,
                    start=True,
                    stop=True,
                    skip_group_check=True,
                )

            # ---------------- per-layer state ----------------
            def mk_tp(Xsrc, ot, ec):
                # transpose 4 bf16 node chunks of Xsrc; the drain converts
                # to fp8 XT (fp8 PE transposes need stride-2 outputs, so
                # transpose in bf16 instead)
                def op():
                    oc0 = ot * OS
                    tp0 = midpool.tile([P, OW], f32, tag="mid")
                    tps = tp0.bitcast(bf16)[:, 0:OW]
                    for j in range(OS):
                        nc.tensor.matmul(
                            tps[:, j * P : (j + 1) * P],
                            lhsT=Xsrc[:, oc0 + j, ec * P : (ec + 1) * P],
                            rhs=identb,
                            is_transpose=True,
                            start=True,
                            stop=True,
                            skip_group_check=True,
                        )
                    nc.scalar.copy(XT[:, ec, ot * OW : (ot + 1) * OW], tps)
                return op

            def mk_proj(k, di, nt):
                # one (dst, node-slice) projection: 4 DR matmuls + drain
                dst, w8, b_sb2 = (
                    (KT, W8K, bk_sb if apply_bias else None),
                    (QT, W8Q, bq_sb if apply_bias else None),
                )[di]

                def op():
                    ps = scpool.tile([P, 2, OW], f32, tag="sc")
                    for fc in range(FC):
                        for e2 in range(2):
                            nc.tensor.matmul(
                                ps[:, fc, :],
                                lhsT=w8[
                                    :, k, 2 * e2 : 2 * e2 + 2,
                                    fc * P : (fc + 1) * P,
                                ],
                                rhs=XT[
                                    :, 2 * e2 : 2 * e2 + 2,
                                    nt * OW : (nt + 1) * OW,
                                ],
                                start=(e2 == 0),
                                stop=(e2 == 1),
                                perf_mode=DR,
                            )
                    dstap = dst[:, k, :, nt * OW : (nt + 1) * OW]
                    if apply_bias:
                        for fc in range(FC):
                            if (di + nt) % 2 == 0:
                                nc.scalar.add(
                                    dstap[:, fc, :],
                                    ps[:, fc, :],
                                    b_sb2[:, k, fc : fc + 1],
                                )
                            else:
                                nc.vector.tensor_scalar(
                                    out=dstap[:, fc, :],
                                    in0=ps[:, fc, :],
                                    scalar1=b_sb2[:, k, fc : fc + 1],
                                    scalar2=None,
                                    op0=ALU.add,
                                )
                    elif (di + nt) % 2 == 0:
                        nc.scalar.copy(dstap, ps)
                    else:
                        nc.vector.tensor_copy(dstap, ps)
                return op

            def projections(k):
                # slice 0 inline; the rest feed phase_a(k, 0)'s interleave
                # so scores can start after just K0/Q0
                mk_proj(k, 0, 0)()
                mk_proj(k, 1, 0)()
                return [
                    mk_proj(k, di, nt)
                    for nt in range(1, OT)
                    for di in range(2)
                ]

            # phase A for one ot: scores+seed -> exp -> dn (dn lags one
            # group so PE never waits on the exp that feeds it)
            def phase_a(k, ot, pT, dn_tile, interleave, drain_by=None,
                        lazy=()):
                osl = slice(ot * OW, (ot + 1) * OW)
                NG = NC // 2
                DB = drain_by or NG
                ii = 0
                jj = 0

                def dn_mm(g):
                    nc.tensor.matmul(
                        dn_tile,
                        lhsT=ones32,
                        rhs=pT[:, 2 * g : 2 * g + 2, :],
                        start=(g == 0),
                        stop=(g == NG - 1),
                        perf_mode=DR,
                    )

                for g in range(NG):
                    sc = scpool.tile([P, 2, OW], f32, tag="sc")
                    for h in range(2):
                        ic = 2 * g + h
                        nc.tensor.matmul(
                            sc[:, h, :],
                            lhsT=KT[:, k, 0:2, ic * P : (ic + 1) * P],
                            rhs=QT[:, k, 0:2, osl],
                            start=True,
                            stop=False,
                            perf_mode=DR,
                        )
                        nc.tensor.matmul(
                            sc[:, h, :],
                            lhsT=seedw,
                            rhs=bcast_pair(CM[:, ot, ic, :]),
                            start=False,
                            stop=True,
                            perf_mode=DR,
                        )
                    nc.scalar.activation(
                        pT[:, 2 * g : 2 * g + 2, :], sc, AF.Exp, scale=SCL
                    )
                    if g > 0:
                        dn_mm(g - 1)
                    # interleave pending ops from prev-ot phases B/C, paced
                    # so the list drains evenly across the first DB groups
                    quota = ((g + 1) * len(interleave) + DB - 1) // DB
                    while ii < min(quota, len(interleave)):
                        interleave[ii]()
                        ii += 1
                    if g >= NG - 3 and lazy:
                        ql = ((g - NG + 4) * len(lazy) + 2) // 3
                        while jj < min(ql, len(lazy)):
                            lazy[jj]()
                            jj += 1
                dn_mm(NG - 1)
                while ii < len(interleave):
                    interleave[ii]()
                    ii += 1
                while jj < len(lazy):
                    lazy[jj]()
                    jj += 1

            # phase B for one ot -> returns list of emission thunks
            def phase_b_ops(k, ot, pT, ctxT8, rb):
                X8 = X8A if k == 0 else X8B

                def mk_ctx(ec):
                    def op():
                        cp = midpool.tile([P, OW], f32, tag="mid")
                        for j in range(NC // 2):
                            nc.tensor.matmul(
                                cp,
                                lhsT=X8[
                                    :, 2 * j : 2 * j + 2,
                                    ec * P : (ec + 1) * P,
                                ],
                                rhs=pT[:, 2 * j : 2 * j + 2, :],
                                start=(j == 0),
                                stop=(j == NC // 2 - 1),
                                perf_mode=DR,
                            )
                        nc.vector.tensor_tensor(
                            ctxT8[:, ec, :], cp, rb[:, :], ALU.mult
                        )
                    return op

                if os.environ.get("ABL_B"):
                    return []
                return [mk_ctx(ec) for ec in range(EC)]

            # phase C for one ot -> list of emission thunks
            def phase_c_ops(k, ot, ctxT8):
                X_in = XA if k == 0 else XB
                X_out = XB if k == 0 else None
                mv = lnpool.tile([P, OS, 2], f32, tag="mv")
                stats = lnpool.tile([P, OS, 6], f32, tag="st")
                y4 = lnpool.tile([P, OS], f32, tag="y4")

                def mk_ctx2(osub):
                    def op():
                        cp = midpool.tile([P, OW], f32, tag="mid")
                        for e2 in range(2):
                            nc.tensor.matmul(
                                cp,
                                lhsT=ctxT8[
                                    :, 2 * e2 : 2 * e2 + 2,
                                    osub * P : (osub + 1) * P,
                                ],
                                rhs=W8C[:, k, 2 * e2 : 2 * e2 + 2, :],
                                start=(e2 == 0),
                                stop=(e2 == 1),
                                perf_mode=DR,
                            )
                        oc = ot * OS + osub
                        # h = ctx2 * RES_S + x  (bf16 staging; DVE --
                        # GPSIMD cannot read the ctx2 PSUM)
                        nc.vector.scalar_tensor_tensor(
                            HB[:, oc, :],
                            cp,
                            RES_S,
                            X_in[:, oc, :],
                            ALU.mult,
                            ALU.add,
                        )
                        # per-chunk LN stats immediately (shortens the tail)
                        nc.vector.bn_stats(stats[:, osub, :], HB[:, oc, :])
                        nc.vector.bn_aggr(mv[:, osub, :], stats[:, osub, :])
                    return op

                def ln_stats():
                    # rstd via Newton on [P, OS] (keeps the dependent
                    # chain off the in-order ACT exp stream)
                    x4 = lnpool.tile([P, OS], f32, tag="x4")
                    t4 = lnpool.tile([P, OS], f32, tag="t4")
                    nc.vector.tensor_scalar_add(x4, mv[:, :, 1], LN_EPS)
                    nc.vector.tensor_scalar(
                        out=y4.bitcast(i32),
                        in0=x4.bitcast(i32),
                        scalar1=1,
                        scalar2=None,
                        op0=ALU.logical_shift_right,
                    )
                    nc.vector.tensor_scalar(
                        out=y4.bitcast(i32),
                        in0=y4.bitcast(i32),
                        scalar1=-1,
                        scalar2=0x5F3759DF,
                        op0=ALU.mult,
                        op1=ALU.add,
                    )
                    for _ in range(2):
                        nc.vector.tensor_mul(t4, y4, y4)
                        nc.vector.tensor_mul(t4, t4, x4)
                        nc.vector.tensor_scalar(
                            out=t4,
                            in0=t4,
                            scalar1=-0.5,
                            scalar2=1.5,
                            op0=ALU.mult,
                            op1=ALU.add,
                        )
                        nc.vector.tensor_mul(y4, y4, t4)

                def mk_apply(osub):
                    def op():
                        oc = ot * OS + osub
                        # x_out (bf16, DVE 4x)
                        nc.vector.tensor_scalar(
                            out=X_out[:, oc, :],
                            in0=HB[:, oc, :],
                            scalar1=mv[:, osub, 0:1],
                            scalar2=y4[:, osub : osub + 1],
                            op0=ALU.subtract,
                            op1=ALU.mult,
                        )
                        if apply_gb:
                            nc.gpsimd.tensor_mul(
                                X_out[:, oc, :], X_out[:, oc, :],
                                g_sb[:, k, :],
                            )
                            nc.gpsimd.tensor_add(
                                X_out[:, oc, :], X_out[:, oc, :],
                                b_sb[:, k, :],
                            )
                            nc.gpsimd.tensor_copy(
                                X8B[:, oc, :], X_out[:, oc, :]
                            )
                        else:
                            nc.gpsimd.tensor_scalar(
                                out=X8B[:, oc, :],
                                in0=HB[:, oc, :],
                                scalar1=mv[:, osub, 0:1],
                                scalar2=y4[:, osub : osub + 1],
                                op0=ALU.subtract,
                                op1=ALU.mult,
                            )
                    return op

                def mk_final(osub):
                    def op():
                        ops_ps = ofpool.tile([P, E], f32, tag="of")
                        oc = ot * OS + osub
                        eng = nc.gpsimd if osub % 2 else nc.vector
                        eng.tensor_scalar(
                            out=ops_ps,
                            in0=HB[:, oc, :],
                            scalar1=mv[:, osub, 0:1],
                            scalar2=y4[:, osub : osub + 1],
                            op0=ALU.subtract,
                            op1=ALU.mult,
                        )
                        if apply_gb:
                            nc.gpsimd.tensor_mul(
                                ops_ps, ops_ps, g_sb[:, k, :]
                            )
                            nc.gpsimd.tensor_add(
                                ops_ps, ops_ps, b_sb[:, k, :]
                            )
                        nc.sync.dma_start(out=outr[:, oc, :], in_=ops_ps)
                    return op

                # c1: consumed during phase_a(ot+1) -- full LN chain so the
                # transposes (c2, phase_a(ot+2)) never stall on applies
                if os.environ.get("ABL_C"):
                    return [], []
                c1 = [mk_ctx2(i) for i in range(OS)] + [ln_stats]
                if k < L - 1:
                    c1 += [mk_apply(i) for i in range(OS)]
                    c2 = ([mk_tp(X_out, ot, ec) for ec in range(EC)]
                          if not os.environ.get("ABL_TP") else [])
                else:
                    c1 += [mk_final(i) for i in range(OS)]
                    c2 = []
                return c1, c2

            # ---------------- main schedule ----------------
            # pipeline: A(k,ot) interleaves B(ot-1) + C1(ot-1) + tp(ot-2);
            # projK(k+1, nt) right after A(k, nt+2); projQ(k+1, nt) lazily
            # just before A(k+1, nt). Pendings carry across the layer
            # boundary (chain-first, forced-early pacing).
            pend = projections(0)
            b_prev = []     # B(ot-1)
            c1_prev = []    # C1(ot-1): ctx2+residual+LN+applies
            c2_p = []       # tp(ot-1)
            c2_pp = []      # tp(ot-2)
            projk_p = []    # projK(k+1, ot-1)
            projk_pp = []   # projK(k+1, ot-2)
            projq = {}      # nt -> projQ(k+1, nt) thunk
            for k in range(L):
                for ot in range(OT):
                    boundary = ot == 0 and k > 0
                    if boundary:
                        # layer tail: ready transposes/projK first, then
                        # the dependent chain; drain early so projK(3)
                        # precedes scores g6 (which reads KT's last slice)
                        inter = (c2_pp + projk_pp + b_prev + c1_prev
                                 + c2_p + projk_p)
                        lz = []
                        c2_pp = c2_p = projk_pp = projk_p = []
                        db = 5
                    else:
                        inter = pend + b_prev + c1_prev + c2_pp
                        lz = []
                        c2_pp = []
                        db = 6 if (k == 0 and ot == 0) else None
                    pend = []
                    if k > 0 and ot in projq:
                        projq.pop(ot)()
                    pT = ptpool.tile([P, NC, OW], f8, tag="pT")
                    dn = dnpool.tile([P, OW], f32, tag="dn")
                    phase_a(k, ot, pT, dn[:, :], inter, drain_by=db,
                            lazy=lz)
                    # projK(k+1, nt=ot-2): its transposes (tp(ot-2)) were
                    # consumed inside phase_a above
                    for op in projk_pp:
                        op()
                    projk_pp = []
                    rb = rbpool.tile([P, OW], f32, tag="rb")
                    nc.vector.reciprocal(rb, dn[:, :])
                    ctxT8 = ctpool.tile([P, EC, OW], f8, tag="ctxT8")
                    b_prev = phase_b_ops(k, ot, pT, ctxT8, rb)
                    c1_prev, c2_new = phase_c_ops(k, ot, ctxT8)
                    c2_pp, c2_p = c2_p, c2_new
                    projk_pp = projk_p
                    if k < L - 1:
                        projk_p = [mk_proj(k + 1, 0, ot)]
                        projq[ot] = mk_proj(k + 1, 1, ot)
                    else:
                        projk_p = []
            # final flush after the last ot of the last layer
            for op in c2_pp + projk_pp + b_prev + c1_prev + c2_p + projk_p:
                op()
    nc.compile()
    return nc


def _get_nc(apply_gb, apply_bias):
    key = ("nc", apply_gb, apply_bias)
    if key not in _CACHE:
        _CACHE[key] = _build(apply_gb, apply_bias)
    return _CACHE[key]


def make_in_maps(inputs, apply_gb=None, apply_bias=None):
    import ml_dtypes

    f8 = ml_dtypes.float8_e4m3
    bf = ml_dtypes.bfloat16

    node_fts = np.asarray(inputs["node_fts"], np.float32)
    rel_edges = np.asarray(inputs["rel_edges"])
    Wq = np.asarray(inputs["Wq"], np.float32)
    bq = np.asarray(inputs["bq"], np.float32)
    Wk = np.asarray(inputs["Wk"], np.float32)
    bk = np.asarray(inputs["bk"], np.float32)
    Wc = np.asarray(inputs["Wc"], np.float32)
    ln_g = np.asarray(inputs["ln_g"], np.float32)
    ln_b = np.asarray(inputs["ln_b"], np.float32)
    if apply_gb is None:
        apply_gb = _needs_gb(inputs)
    if apply_bias is None:
        apply_bias = _needs_bias(inputs)

    wq8 = np.ascontiguousarray(
        (WS * Wq).transpose(0, 2, 1)).astype(f8)          # [L, E, FF]
    wk8 = np.ascontiguousarray((WS * Wk).transpose(0, 2, 1)).astype(f8)
    wc8 = np.ascontiguousarray((WS * Wc).transpose(0, 2, 1)).astype(f8)
    idn8 = np.eye(P, dtype=np.float32).astype(f8)
    seedw = np.zeros((P, 2 * P), np.float32)
    seedw[:, :P] = -MK * np.eye(P)
    seedw = seedw.astype(f8)
    ones = np.full((P, 2 * P), 1.0 / CTX_S, np.float32).astype(f8)

    in_maps = []
    for c in range(B):
        x = node_fts[c]
        # cm8[p, ot, ic, j] = MK where no edge(o <- i), i=ic*128+p, o=ot*512+j
        noedge = (rel_edges[c] == 0)                       # [o, i]
        cm = noedge.T.reshape(NC, P, OT, OW).transpose(1, 2, 0, 3)
        cm8 = np.ascontiguousarray(
            cm.astype(np.float32) * MK).astype(f8).reshape(P, -1)
        m = {
            "x0bf": np.ascontiguousarray(x).astype(bf),
            "x08": np.ascontiguousarray(x).astype(f8),
            "x0t8": np.ascontiguousarray(x.T).astype(f8),
            "wq8": wq8,
            "wk8": wk8,
            "wc8": wc8,
            "cm8": cm8,
            "idn8": idn8,
            "idnb": np.eye(P, dtype=np.float32).astype(bf),
            "seedwd": seedw,
            "onesd": ones,
        }
        if apply_bias:
            m["bqd"] = WS * bq
            m["bkd"] = WS * bk
        if apply_gb:
            m["ln_g"] = ln_g
            m["ln_b"] = ln_b
        in_maps.append(m)
    return in_maps


def _needs_gb(inputs):
    g = np.asarray(inputs["ln_g"], np.float32)
    b = np.asarray(inputs["ln_b"], np.float32)
    return not (np.all(g == 1.0) and np.all(b == 0.0))


def _needs_bias(inputs):
    bq = np.asarray(inputs["bq"], np.float32)
    bk = np.asarray(inputs["bk"], np.float32)
    return not (np.all(bq == 0.0) and np.all(bk == 0.0))


def kernel(**inputs) -> np.ndarray:
    from concourse.bass_utils import run_bass_kernel_spmd

    apply_gb = _needs_gb(inputs)
    apply_bias = _needs_bias(inputs)
    nc = _get_nc(apply_gb, apply_bias)
    in_maps = make_in_maps(inputs, apply_gb, apply_bias)
    res = run_bass_kernel_spmd(nc, in_maps, core_ids=list(range(B)))
    return np.stack([r["outp"] for r in res.results], axis=0)
